# revision 8
# baseline (speedup 1.0000x reference)
"""GQA attention (16 q-heads / 4 kv-heads, head_dim 64, T=2048, D=1024) on 8
Trainium2 NeuronCores.

Sharding: 8 shards = batch(2) x kv-group(4). Each core handles one batch
element and one whole GQA group (4 query heads + their shared kv head), and
computes a partial output projection; the host sums the 4 group-partials per
batch element.

v3: single round-robin emission interleaving projection (B), attention (C)
and output-projection (D) work so the scalar-engine exp stream overlaps
PE/vector work from neighboring phases. Head-pair row-tiled scores, per-kt
fused 2-head exp, triangle-only causal masking on gpsimd, approx-reciprocal
softmax denominators, bf16 staged output.

Self-contained: hardcodes all shapes; inputs are the full unsharded tensors.
"""
import sys

if "/opt/trn_rl_repo" not in sys.path:
    sys.path.insert(0, "/opt/trn_rl_repo")

import numpy as np
import ml_dtypes

T = 2048
D = 1024
HD = 64
NH = 4          # q heads per core
TT = 16         # t-tiles of 128
W = 512         # q-chunk width in attention
NCH = 4         # number of q-chunks
EPS = 1e-6

_CACHE = {}


def _build_nc(variant="full"):
    import concourse.bass as bass
    import concourse.tile as tile
    from concourse import bacc, mybir
    from concourse.masks import make_identity

    F32 = mybir.dt.float32
    BF16 = mybir.dt.bfloat16
    AF = mybir.ActivationFunctionType
    AX = mybir.AxisListType

    nc = bacc.Bacc("TRN2", target_bir_lowering=False, debug=False,
                   num_devices=8)

    xT_d = nc.dram_tensor("xT", [D, T], BF16, kind="ExternalInput")
    wqkvT_d = nc.dram_tensor("wqkvT", [D, 384], BF16, kind="ExternalInput")
    woT_d = nc.dram_tensor("woT", [256, D], BF16, kind="ExternalInput")
    cc_d = nc.dram_tensor("cc", [T, 320], BF16, kind="ExternalInput")
    ss_d = nc.dram_tensor("ss", [T, 320], BF16, kind="ExternalInput")
    out_d = nc.dram_tensor("out", [T, D], BF16, kind="ExternalOutput")

    with tile.TileContext(nc) as tc:
        with tc.tile_pool(name="singles", bufs=1) as singles:
            # --- persistent SBUF tensors ---
            ident = singles.tile([128, 128], BF16)
            make_identity(nc, ident)
            ones = singles.tile([128, 64], BF16)
            nc.vector.memset(ones, 1.0)
            # triangle mask M[p, c] = 1.0 if p <= c else 0.0 (c over 128)
            M = singles.tile([128, 128], BF16)
            nc.gpsimd.memset(M, 1.0)
            nc.gpsimd.affine_select(
                out=M, in_=M, compare_op=mybir.AluOpType.is_ge, fill=0.0,
                base=0, channel_multiplier=-1, pattern=[[1, 128]])

            wqkv = singles.tile([128, 8, 384], BF16)
            nc.sync.dma_start(
                out=wqkv, in_=wqkvT_d[:, :].rearrange("(c p) n -> p c n", p=128))
            wo = singles.tile([128, 2, 1024], BF16)
            nc.sync.dma_start(
                out=wo, in_=woT_d[:, :].rearrange("(c p) n -> p c n", p=128))

            xT = []
            for c in range(8):
                xt = singles.tile([128, T], BF16, tag=f"xT{c}")
                nc.sync.dma_start(
                    out=xt, in_=xT_d[128 * c:128 * (c + 1), :])
                xT.append(xt)

            qT = singles.tile([128, 2, T], BF16)       # 4 heads: (h%2)*64+dh, h//2
            kT = singles.tile([128, T], BF16)          # kv head dims duplicated 2x
            v1 = singles.tile([128, TT, 65], BF16)     # V tiles + ones col
            nc.vector.memset(v1[:, :, 64:65], 1.0)
            attnT = singles.tile([128, 2, T], BF16)    # normalized attn out^T
            # es_all[p, kt, pair, slot, 512] persistent exp(scores) buffers
            es_all = singles.tile([128, TT, 2, 2, W], BF16)
            eps_t = singles.tile([128, 1], F32)
            nc.vector.memset(eps_t, EPS)

            with tc.tile_pool(name="ps_sc", bufs=1, space="PSUM") as ps_sc, \
                 tc.tile_pool(name="ps_pv", bufs=1, space="PSUM") as ps_pv, \
                 tc.tile_pool(name="ps_w", bufs=2, space="PSUM") as ps_w, \
                 tc.tile_pool(name="bwork", bufs=3) as bwork, \
                 tc.tile_pool(name="bsmall", bufs=3) as bsmall, \
                 tc.tile_pool(name="cwork", bufs=2) as cwork, \
                 tc.tile_pool(name="dwork", bufs=3) as dwork:

                def emit_b_tile(i):
                    ts = slice(128 * i, 128 * (i + 1))
                    pq = ps_w.tile([128, 384], F32, tag="w", name="pq")
                    for c in range(8):
                        nc.tensor.matmul(
                            pq, xT[c][:, ts], wqkv[:, c, :],
                            start=(c == 0), stop=(c == 7))
                    nc.scalar.copy(v1[:, i, 0:64], pq[:, 320:384])
                    # rmsnorm: square on scalar (single-PSUM-operand rule)
                    sq = bwork.tile([128, 320], F32, tag="sq")
                    nc.scalar.activation(out=sq, in_=pq[:, 0:320],
                                         func=AF.Square)
                    ssum = bsmall.tile([128, 5], F32, tag="ssum")
                    nc.vector.reduce_sum(
                        out=ssum, in_=sq.rearrange("p (h d) -> p h d", h=5),
                        axis=AX.X)
                    stdv = bsmall.tile([128, 5], F32, tag="stdv")
                    nc.scalar.activation(
                        out=stdv, in_=ssum, func=AF.Sqrt, bias=eps_t[:, :],
                        scale=1.0 / HD)
                    rstd = bsmall.tile([128, 5], F32, tag="rstd")
                    nc.vector.reciprocal(rstd, stdv)
                    # one broadcast multiply: qkb = pq * rstd (bf16 out)
                    qkb = bwork.tile([128, 320], BF16, tag="qkb")
                    rstd_b = bass.AP(
                        tensor=rstd.tensor, offset=rstd.offset,
                        ap=[rstd.ap[0], [rstd.ap[-1][0], 5], [0, 64]])
                    nc.vector.tensor_mul(
                        qkb.rearrange("p (h d) -> p h d", h=5),
                        pq[:, 0:320].rearrange("p (h d) -> p h d", h=5),
                        rstd_b)
                    # rope in bf16
                    cs = bwork.tile([128, 320], BF16, tag="cs")
                    nc.sync.dma_start(out=cs, in_=cc_d[ts, :])
                    sn = bwork.tile([128, 320], BF16, tag="sn")
                    nc.sync.dma_start(out=sn, in_=ss_d[ts, :])
                    xc = bwork.tile([128, 320], BF16, tag="xc")
                    nc.vector.tensor_mul(xc, qkb, cs)
                    qkb_swap = bass.AP(
                        tensor=qkb.tensor, offset=qkb.offset + 32,
                        ap=[qkb.ap[0], [64, 5], [-32, 2], [1, 32]])
                    xs = bwork.tile([128, 5, 64], BF16, tag="xs")
                    nc.vector.tensor_mul(xs, qkb_swap, sn.rearrange(
                        "p (h d) -> p h d", h=5))
                    rope = bwork.tile([128, 320], BF16, tag="rope")
                    nc.vector.tensor_add(
                        rope, xc, xs.rearrange("p h d -> p (h d)"))
                    ktr = bwork.tile([128, 128], BF16, tag="ktr")
                    nc.gpsimd.tensor_copy(ktr[:, 0:64], rope[:, 256:320])
                    nc.gpsimd.tensor_copy(ktr[:, 64:128], rope[:, 256:320])
                    for pair in range(2):
                        tp = ps_w.tile([128, 128], BF16, tag="w", name="tp")
                        nc.tensor.transpose(
                            tp, rope[:, 128 * pair:128 * (pair + 1)],
                            ident)
                        if pair == 0:
                            nc.scalar.copy(qT[:, 0, ts], tp)
                        else:
                            nc.vector.tensor_copy(qT[:, 1, ts], tp)
                    tpk = ps_w.tile([128, 128], BF16, tag="w", name="tpk")
                    nc.tensor.transpose(tpk, ktr, ident)
                    nc.vector.tensor_copy(kT[:, ts], tpk)

                def emit_c_kt(j, kt):
                    """scores + exp (+mask) for one k-tile of chunk j, then
                    PV accumulation for all 4 heads."""
                    qs = slice(W * j, W * (j + 1))
                    nkt = 4 * j + 4
                    kts = slice(128 * kt, 128 * (kt + 1))
                    delta = 128 * kt - W * j
                    lo = max(0, delta)
                    for hc in range(2):
                        sc2 = ps_sc.tile([128, 2, W], F32, tag="sc2")
                        nc.tensor.matmul(
                            sc2[:, 0, :], kT[0:64, kts],
                            qT[0:64, hc, qs], start=True, stop=True)
                        nc.tensor.matmul(
                            sc2[:, 1, :], kT[64:128, kts],
                            qT[64:128, hc, qs], start=True, stop=True)
                        es2 = es_all[:, kt, hc, :, :]
                        if lo > 0:
                            nc.gpsimd.memset(es2[:, :, 0:lo], 0.0)
                        nc.scalar.activation(
                            out=es2[:, :, lo:W], in_=sc2[:, :, lo:W],
                            func=AF.Exp)
                        if delta >= 0:  # diagonal block: mask triangle
                            import concourse.bass as bass_mod
                            M_b = bass_mod.AP(
                                tensor=M.tensor, offset=M.offset,
                                ap=[M.ap[0], [0, 2], [1, 128]])
                            nc.gpsimd.tensor_mul(
                                es2[:, :, lo:lo + 128],
                                es2[:, :, lo:lo + 128], M_b)
                    pv4 = _round_state["pv4"]
                    for h in range(NH):
                        nc.tensor.matmul(
                            pv4[:, h, :], v1[:, kt, :],
                            es_all[:, kt, h // 2, h % 2, :],
                            start=(kt == 0), stop=(kt == nkt - 1))

                def emit_c3(j):
                    qs = slice(W * j, W * (j + 1))
                    pv4 = _round_state["pv4"]
                    sumsb = cwork.tile([1, NH, W], BF16, tag="sumsb")
                    nc.vector.tensor_copy(sumsb, pv4[64:65, :, :])
                    for h in range(NH):
                        bcp = ps_w.tile([64, W], F32, tag="w", name="bcp")
                        nc.tensor.matmul(
                            bcp, ones[0:1, 0:64], sumsb[0:1, h, :],
                            start=True, stop=True)
                        rcb = cwork.tile([64, W], F32, tag="rcb")
                        nc.vector.reciprocal_approx_fast(out=rcb, in_=bcp)
                        hp = 64 * (h % 2)
                        nc.vector.tensor_mul(
                            attnT[hp:hp + 64, h // 2, qs],
                            pv4[0:64, h, :], rcb)

                def emit_d_tile(i):
                    ts = slice(128 * i, 128 * (i + 1))
                    ob = dwork.tile([128, 1024], BF16, tag="ob")
                    for nh in range(2):
                        po = ps_w.tile([128, 512], F32, tag="w", name="po")
                        for c in range(2):
                            nc.tensor.matmul(
                                po, attnT[:, c, ts],
                                wo[:, c, 512 * nh:512 * (nh + 1)],
                                start=(c == 0), stop=(c == 1))
                        if nh == 0:
                            nc.scalar.copy(ob[:, 0:512], po)
                        else:
                            nc.vector.tensor_copy(ob[:, 512:1024], po)
                    nc.sync.dma_start(out=out_d[ts, :], in_=ob)

                # ---------------- round-robin emission -----------------------
                _round_state = {}
                for i in range(4):
                    emit_b_tile(i)
                d_queue = []
                for r in range(1, 5):
                    j = r - 1
                    nkt = 4 * j + 4
                    _round_state["pv4"] = ps_pv.tile(
                        [65, NH, W], F32, tag="pv4", name="pv4")
                    fill = ([("b", 4 * r + t) for t in range(4)] if r <= 3
                            else [])
                    fill += d_queue
                    d_queue = []
                    fi = 0
                    for kt in range(nkt):
                        emit_c_kt(j, kt)
                        # interleave one B/D item every other kt
                        if kt % 2 == 1 and fi < len(fill):
                            kind, idx = fill[fi]; fi += 1
                            (emit_b_tile if kind == "b" else emit_d_tile)(idx)
                    emit_c3(j)
                    while fi < len(fill):
                        kind, idx = fill[fi]; fi += 1
                        (emit_b_tile if kind == "b" else emit_d_tile)(idx)
                    d_queue = [("d", 4 * j + t) for t in range(4)]
                for kind, idx in d_queue:
                    emit_d_tile(idx)

            if variant == "attn":
                nc.gpsimd.dma_start(out=out_d[0:128, 0:D],
                                    in_=attnT[:, 0, 0:D])
    nc.compile()
    return nc


def _host_tables(cos, sin, qn_w, kn_w):
    scale = HD ** -0.5
    cch = np.concatenate([cos, cos], 1).astype(np.float32)         # (T, 64)
    ssh = np.concatenate([-sin, sin], 1).astype(np.float32)
    qn4 = np.tile(qn_w, 4).astype(np.float32)
    swq4 = np.tile(np.concatenate([qn_w[32:], qn_w[:32]]), 4).astype(np.float32)
    swk = np.concatenate([kn_w[32:], kn_w[:32]]).astype(np.float32)
    cc = np.concatenate(
        [np.tile(cch, (1, NH)) * qn4[None] * scale, cch * kn_w[None]], 1)
    ss = np.concatenate(
        [np.tile(ssh, (1, NH)) * swq4[None] * scale, ssh * swk[None]], 1)
    return (np.ascontiguousarray(cc.astype(ml_dtypes.bfloat16)),
            np.ascontiguousarray(ss.astype(ml_dtypes.bfloat16)))


def make_in_maps(x, cos, sin, wq, wk, wv, wo, qn_w, kn_w):
    cc, ss = _host_tables(cos, sin, qn_w, kn_w)
    in_maps = []
    for core in range(8):
        b, g = divmod(core, 4)
        wqkvT = np.ascontiguousarray(np.concatenate(
            [wq[256 * g:256 * (g + 1)],
             wk[64 * g:64 * (g + 1)],
             wv[64 * g:64 * (g + 1)]], 0).T.astype(ml_dtypes.bfloat16))
        woT = np.ascontiguousarray(
            wo[:, 256 * g:256 * (g + 1)].T.astype(ml_dtypes.bfloat16))
        xT = np.ascontiguousarray(np.asarray(x)[b].T.astype(ml_dtypes.bfloat16))
        in_maps.append({"xT": xT, "wqkvT": wqkvT, "woT": woT,
                        "cc": cc, "ss": ss})
    return in_maps


def kernel(x, cos, sin, wq, wk, wv, wo, qn_w, kn_w):
    from concourse.bass_utils import run_bass_kernel_spmd

    if "nc" not in _CACHE:
        _CACHE["nc"] = _build_nc()
    nc = _CACHE["nc"]
    in_maps = make_in_maps(np.asarray(x), np.asarray(cos), np.asarray(sin),
                           np.asarray(wq), np.asarray(wk), np.asarray(wv),
                           np.asarray(wo), np.asarray(qn_w), np.asarray(kn_w))
    res = run_bass_kernel_spmd(nc, in_maps, core_ids=list(range(8)))
    out = np.zeros((2, T, D), np.float32)
    for core in range(8):
        b = core // 4
        out[b] += res.results[core]["out"].astype(np.float32)
    return out
